# revision 27
# baseline (speedup 1.0000x reference)
"""Trainium2 Bass kernel for the capsule-routing layer (nn_Caps_Layer).

Computation (per batch b of x [B, S, D], W [D, 25]):
  u_hat = (x_b @ W).reshape(S, 5, 5)           # [S, n, k]
  b0 = 0;  for 4 routing iters:
    c = softmax_n(b)                            # over the 5 capsules
    v[n,k] = sum_s c[n,s] u_hat[s,n,k]
    out = v / sqrt(sum_k v^2 + 1e-7)
    b[n,s] = sum_k out[n,k] u_hat[s,n,k]
Returns out [B, 5, 5].

Sharding: pure data-parallel over batch across 8 NeuronCores (16 batches
each); W replicated; no collectives.

Staging: the host feeds x pre-transposed per batch and cast to fp16 with
layout [b, p, db, s] (each SBUF partition line is one contiguous 6 KB
DMA chunk).  The device then needs NO PE transposes:

  per batch: DMA xT [128 d_lo, (db, s)] fp16; for (sh, db): matmul with
    the xT 128x128 chunk stationary and W[db] [128, 25] moving ->
    u_hat natural [128 s_lo, 25] accumulated over db in PSUM.
  per group of G batches: one PSUM->SBUF fp16 copy -> uh [128, G*4*25].

Routing (per group, natural layout): the s-partition sum AND the
broadcast of v to all partitions is ONE ones[128,128] matmul (pv128);
the squash norm runs replicated on all 128 partitions, with
rsqrt = Exp(-0.5*Ln(cs^2*s2 + eps) + ln(cs)) so the ACT engine only
ever uses the {exp, ln, square, copy} table set (zero table reloads).
Routing iterations of different groups are emitted wave- and
step-interleaved so their latency chains overlap on every engine.
"""

from contextlib import ExitStack
import math

import numpy as np

import concourse.bass as bass
import concourse.tile as tile
from concourse import mybir

F32 = mybir.dt.float32
F32R = mybir.dt.float32r
F16 = mybir.dt.float16
BF16 = mybir.dt.bfloat16
AX = mybir.AxisListType
OP = mybir.AluOpType
AF = mybir.ActivationFunctionType

N_CORES = 8
B_FULL, S, D = 128, 512, 768
NCAP, KDIM = 5, 5
NK = NCAP * KDIM  # 25
ROUTINGS = 4
T_EPS = 1e-7

ND = D // 128   # 6 d-blocks
NSB = S // 128  # 4 s-blocks (= s_hi)


class GroupState:
    def __init__(self, boff, G, pu):
        self.boff = boff
        self.G = G
        self.pu = pu
        self.uh16 = None  # fp16 SBUF copy of u_hat, made by iter0's s_t0
        self.blog = None  # logits tile, set by each iter for the next


def emit(ctx, tc, out, x, w, b_loc=16, group=4, warm=20):
    """Emit the per-core kernel IR.

    out: [1, b_loc*25] f32; x: [b_loc*128, 6*512] fp16 (pre-transposed,
    partition-major); w: [128, 6*25] fp16 (d_lo major, (db, nk) free).
    """
    nc = tc.nc
    groups = list(group) if isinstance(group, (list, tuple)) else \
        [group] * (b_loc // group)
    assert sum(groups) == b_loc
    NG = len(groups)

    const_pool = ctx.enter_context(tc.tile_pool(name="const", bufs=1))
    xt_pool = ctx.enter_context(tc.tile_pool(name="xt", bufs=16))
    pu_pool = ctx.enter_context(tc.tile_pool(name="pu", bufs=2, space="PSUM"))
    rt_pool = ctx.enter_context(tc.tile_pool(name="rt", bufs=3))
    pv_pool = ctx.enter_context(tc.tile_pool(name="pv", bufs=4, space="PSUM"))

    # --- first x DMAs before anything else on the sync ring ---
    xt_tiles = {}
    def emit_batch_dma(b):
        xt = xt_pool.tile([128, ND * S], F16, tag="xt", name=f"xt{b}")
        eng = nc.sync if (b % 2 == 0) else nc.scalar
        eng.dma_start(xt[:], x[b * 128:(b + 1) * 128, :])
        xt_tiles[b] = xt

    # --- constants (W first: every main matmul needs it) ---
    w_sb = const_pool.tile([128, ND * NK], F16)
    nc.sync.dma_start(w_sb[:], w)
    for _b in range(b_loc):
        emit_batch_dma(_b)
    ones_f = const_pool.tile([128, 128], F32)
    nc.gpsimd.memset(ones_f[:], 1.0)
    ones_sq = const_pool.tile([128, 128], F16)
    nc.vector.tensor_copy(ones_sq[:], ones_f[:])
    eps_b = const_pool.tile([128, 1], F32)
    nc.gpsimd.memset(eps_b[:], T_EPS)
    lncs_b = const_pool.tile([128, 1], F32)
    nc.gpsimd.memset(lncs_b[:], math.log(1.0 / NCAP))

    # Trigger the single ACT table load ({exp, ln, square, copy} set)
    # during the first DMA rather than on the routing critical path.
    dexp = const_pool.tile([1, 1], F32)
    nc.scalar.activation(dexp[:], ones_f[0:1, 0:1], AF.Exp)

    # HAM warm-up: regular fp16 matmuls during the first DMA so the PE
    # clock gate is at 8/8 when real work lands.
    wps = pv_pool.tile([1, 128], F32, tag="warm", bufs=1)
    for _ in range(warm):
        nc.tensor.matmul(wps[:], ones_sq[:, 0:1], ones_sq[:], start=True, stop=True)

    def emit_batch_mms(pu, bi, b):
        if b not in xt_tiles:
            emit_batch_dma(b)
        xt = xt_tiles.pop(b)
        for sh in range(NSB):
            col = (bi * NSB + sh) * NK
            for db in range(ND):
                nc.tensor.matmul(
                    pu[:, col:col + NK],
                    xt[:, db * S + sh * 128:db * S + sh * 128 + 128],
                    w_sb[:, db * NK:(db + 1) * NK],
                    start=(db == 0),
                    stop=(db == ND - 1),
                )

    def routing_iter_steps(st, it):
        """Step closures for one routing iteration of one group.

        Engine split per iter: ACT {exp, square, ln, exp(rsqrt)},
        DVE {den-reduce, recip, c, t, tmp, blog}, Pool {s2},
        PE {4 accumulating pv128 matmuls}.
        """
        G = st.G
        cs = 1.0 / NCAP if it == 0 else 1.0

        def uh_ap():
            return st.uh16[:].rearrange(
                "p (b sh n k) -> p b sh n k", b=G, sh=NSB, n=NCAP
            )
        steps = []
        box = {}

        if it > 0:
            def s_exp(blog=st.blog):
                expb = rt_pool.tile([128, G * NSB * NCAP], BF16, tag="expb")
                nc.scalar.activation(expb[:], blog[:], AF.Exp)
                box["expb"] = expb
            steps.append(s_exp)

            def s_den():
                den = rt_pool.tile([128, G * NSB], F32, tag="den")
                nc.vector.reduce_sum(
                    den[:],
                    box["expb"][:].rearrange("p (bs n) -> p bs n", n=NCAP),
                    axis=AX.X,
                )
                rden = rt_pool.tile([128, G * NSB], F32, tag="rden")
                nc.vector.reciprocal(rden[:], den[:])
                box["rden"] = rden
            steps.append(s_den)

            def s_c():
                c = rt_pool.tile([128, G * NSB * NCAP], F16, tag="c")
                nc.gpsimd.tensor_tensor(
                    c[:].rearrange("p (b sh n) -> p b sh n", b=G, sh=NSB),
                    box["expb"][:].rearrange("p (b sh n) -> p b sh n", b=G, sh=NSB),
                    box["rden"][:].rearrange("p (b sh) -> p b sh", b=G)
                    .unsqueeze(3)
                    .broadcast_to((128, G, NSB, NCAP)),
                    op=OP.mult,
                )
                box["c"] = c
            steps.append(s_c)

            def s_t():
                t = rt_pool.tile([128, G * NSB * NK], F16, tag="t")
                t_ap = t[:].rearrange(
                    "p (b sh n k) -> p b sh n k", b=G, sh=NSB, n=NCAP
                )
                nc.vector.tensor_tensor(
                    t_ap,
                    uh_ap(),
                    box["c"][:]
                    .rearrange("p (b sh n) -> p b sh n", b=G, sh=NSB)
                    .unsqueeze(4)
                    .broadcast_to((128, G, NSB, NCAP, KDIM)),
                    op=OP.mult,
                )
                box["t_ap"] = t_ap
            steps.append(s_t)
        else:
            def s_t0():
                uh16 = rt_pool.tile(
                    [128, G * NSB * NK], F16, tag="uh", bufs=5,
                    name=f"uh{st.boff}",
                )
                nc.vector.tensor_copy(uh16[:], st.pu[:])
                st.uh16 = uh16
                box["t_ap"] = uh16[:].rearrange(
                    "p (b sh n k) -> p b sh n k", b=G, sh=NSB, n=NCAP
                )
            steps.append(s_t0)

        def s_mm():
            # v replicated to all 128 partitions: ones[128,128] stationary,
            # accumulate the 4 s_hi blocks.
            pv = pv_pool.tile([128, G * NK], F32, tag="pv")
            for sh in range(NSB):
                nc.tensor.matmul(
                    pv[:],
                    ones_sq[:],
                    box["t_ap"][:, :, sh, :, :],
                    start=(sh == 0),
                    stop=(sh == NSB - 1),
                )
            box["pv"] = pv
        steps.append(s_mm)

        def s_sq():
            sq = rt_pool.tile([128, G * NK], F32, tag="sq")
            nc.scalar.activation(sq[:], box["pv"][:], AF.Square)
            box["sq"] = sq
        steps.append(s_sq)

        def s_s2():
            s2 = rt_pool.tile([128, G * NCAP], F32, tag="s2")
            nc.vector.reduce_sum(
                s2[:],
                box["sq"][:].rearrange("p (bn k) -> p bn k", k=KDIM),
                axis=AX.X,
            )
            box["s2"] = s2
        steps.append(s_s2)

        def s_rnrm():
            # rsqrt via the exp/ln table set: cs/sqrt(cs^2*s2 + eps)
            lns = rt_pool.tile([128, G * NCAP], F32, tag="lns")
            nc.scalar.activation(
                lns[:], box["s2"][:], AF.Ln, bias=eps_b[:], scale=cs * cs
            )
            rnrm = rt_pool.tile([128, G * NCAP], F32, tag="rnrm")
            if it == 0:
                nc.scalar.activation(
                    rnrm[:], lns[:], AF.Exp, bias=lncs_b[:], scale=-0.5
                )
            else:
                nc.scalar.activation(rnrm[:], lns[:], AF.Exp, scale=-0.5)
            box["rnrm"] = rnrm
        steps.append(s_rnrm)

        if it < ROUTINGS - 1:
            def s_tmp():
                tmp = rt_pool.tile([128, G * NSB * NK], F32, tag="tmp")
                nc.vector.tensor_tensor(
                    tmp[:].rearrange(
                        "p (b sh n k) -> p b sh n k", b=G, sh=NSB, n=NCAP
                    ),
                    uh_ap(),
                    box["pv"][:]
                    .rearrange("p (b n k) -> p b n k", b=G, n=NCAP)
                    .unsqueeze(2)
                    .broadcast_to((128, G, NSB, NCAP, KDIM)),
                    op=OP.mult,
                )
                box["tmp"] = tmp
            steps.append(s_tmp)

            def s_wt():
                wt = rt_pool.tile([128, G * NSB * NCAP], F32, tag="wt")
                nc.vector.reduce_sum(
                    wt[:],
                    box["tmp"][:].rearrange("p (bsn k) -> p bsn k", k=KDIM),
                    axis=AX.X,
                )
                box["wt"] = wt
            steps.append(s_wt)

            def s_blog():
                blog = rt_pool.tile([128, G * NSB * NCAP], F32, tag="blog")
                nc.gpsimd.tensor_tensor(
                    blog[:].rearrange("p (b sh n) -> p b sh n", b=G, sh=NSB),
                    box["wt"][:].rearrange("p (b sh n) -> p b sh n", b=G, sh=NSB),
                    box["rnrm"][:]
                    .rearrange("p (b n) -> p b n", b=G)
                    .unsqueeze(2)
                    .broadcast_to((128, G, NSB, NCAP)),
                    op=OP.mult,
                )
                st.blog = blog
            steps.append(s_blog)
        else:
            def s_out():
                outs = rt_pool.tile([1, G * NK], F32, tag="outs")
                nc.vector.tensor_tensor(
                    outs[:].rearrange("p (b n k) -> p b n k", b=G, n=NCAP),
                    box["pv"][0:1, :].rearrange("p (b n k) -> p b n k", b=G, n=NCAP),
                    box["rnrm"][0:1, :]
                    .rearrange("p (b n) -> p b n", b=G)
                    .unsqueeze(3)
                    .broadcast_to((1, G, NCAP, KDIM)),
                    op=OP.mult,
                )
                nc.sync.dma_start(
                    out[0:1, st.boff * NK:(st.boff + G) * NK], outs[0:1, :]
                )
            steps.append(s_out)
        return steps

    def emit_group_loads(boff, G):
        """Main matmuls for one group; u_hat accumulates in PSUM (pu).

        Each batch's matmuls are stamped with the batch's estimated DMA
        arrival time so the Tile scheduler (which does not model the
        DMA-paced reality) interleaves the per-engine streams correctly.
        """
        pu = pu_pool.tile([128, G * NSB * NK], F32, tag="pu", name=f"pu{boff}")
        for bi in range(G):
            b = boff + bi
            with tc.tile_wait_until(arrival(b)):
                emit_batch_mms(pu, bi, b)
        return GroupState(boff, G, pu)

    def arrival(b):
        # est. µs (as ms) when batch b's DMA completes: start ~10µs,
        # ~2.4µs of wire time per 786KB batch
        return (12.4 + 2.36 * b) / 1000.0

    # --- sequential emission; tile_wait_until stamps drive the global
    # software pipeline inside the Tile scheduler ---
    chains = []
    boff = 0
    for G in groups:
        chains.append(emit_group_loads(boff, G))
        boff += G
    for st in chains:
        uh_ts = arrival(st.boff + st.G - 1) + 0.8 / 1000.0
        chain_est = (1.8 + 0.35 * st.G) / 1000.0
        for it in range(ROUTINGS):
            with tc.tile_wait_until(uh_ts + it * chain_est):
                for fn in routing_iter_steps(st, it):
                    fn()


def legalize_waits(nc):
    """This toolchain's walrus codegen accepts at most ONE sync wait per
    instruction ("Too many sync wait commands" otherwise) — and PE Matmult
    appears to take none safely. Hoist excess waits onto wait-only
    EventSemaphore instructions inserted just before, on the same engine
    (same pattern walrus already accepts for Tile's engine barriers)."""
    n = 0
    for fn in nc.m.functions:
        for blk in fn.blocks:
            new = []
            for inst in blk.instructions:
                si = inst.sync_info
                if si is not None and len(si.on_wait) > 0:
                    waits = list(si.on_wait)
                    keep = 0 if type(inst).__name__ == "InstMatmult" else 1
                    if len(waits) > keep:
                        for wt in waits[: len(waits) - keep]:
                            ev = mybir.InstEventSemaphore(
                                name=f"I-waitfix-{nc.next_id()}"
                            )
                            ev.engine = inst.engine
                            ev.sync_info = mybir.SyncInfo(on_wait=[wt], on_update=[])
                            new.append(ev)
                            n += 1
                        si.on_wait = waits[len(waits) - keep:]
                new.append(inst)
            blk.instructions = new
    return n


def build_caps_kernel(b_loc=16, group=4, warm=20):
    nc = bass.Bass(trn_type="TRN2", debug=False, target_bir_lowering=False)
    x = nc.dram_tensor("x", [b_loc * 128, ND * S], F16, kind="ExternalInput").ap()
    w = nc.dram_tensor("w", [128, ND * NK], F16, kind="ExternalInput").ap()
    out = nc.dram_tensor("out", [1, b_loc * NK], F32, kind="ExternalOutput").ap()
    with tile.TileContext(nc) as tc:
        with ExitStack() as ctx:
            emit(ctx, tc, out, x, w, b_loc=b_loc, group=group, warm=warm)
    legalize_waits(nc)
    return nc


_KERNEL_CFG = dict(group=(4, 4, 4, 2, 2), warm=20)


def prepare_in_maps(x: np.ndarray, W: np.ndarray, b_loc: int):
    """Shard + stage inputs: per core, x transposed to d-major per batch,
    laid out [b, p, db, s] (6 KB contiguous per partition) in fp16;
    W rearranged to [128, (db, nk)] fp16."""
    w16 = np.ascontiguousarray(
        W.reshape(ND, 128, NK).transpose(1, 0, 2).reshape(128, ND * NK)
    ).astype(np.float16)
    maps = []
    for i in range(N_CORES):
        xs = x[i * b_loc:(i + 1) * b_loc]  # [b_loc, S, D] f32
        # [b, S, D] -> [b, D, S] -> [b, db, p, s] -> [b, p, db, s]
        xt = (
            xs.transpose(0, 2, 1)
            .reshape(b_loc, ND, 128, S)
            .transpose(0, 2, 1, 3)
            .astype(np.float16)
        )
        maps.append({
            "x": np.ascontiguousarray(xt).reshape(b_loc * 128, ND * S),
            "w": w16,
        })
    return maps


def kernel(x: np.ndarray, W: np.ndarray) -> np.ndarray:
    from concourse.bass_utils import run_bass_kernel_spmd

    B, S_, D_ = x.shape
    assert (B, S_, D_) == (B_FULL, S, D)
    b_loc = B // N_CORES
    nc = build_caps_kernel(b_loc=b_loc, **_KERNEL_CFG)
    in_maps = prepare_in_maps(x, W, b_loc)
    res = run_bass_kernel_spmd(nc, in_maps, core_ids=list(range(N_CORES)))
    outs = [res.results[i]["out"].reshape(b_loc, NCAP, KDIM) for i in range(N_CORES)]
    return np.concatenate(outs, axis=0).astype(np.float32)


# revision 28
# speedup vs baseline: 1.1548x; 1.1548x over previous
"""Trainium2 Bass kernel for the capsule-routing layer (nn_Caps_Layer).

Computation (per batch b of x [B, S, D], W [D, 25]):
  u_hat = (x_b @ W).reshape(S, 5, 5)           # [S, n, k]
  b0 = 0;  for 4 routing iters:
    c = softmax_n(b)                            # over the 5 capsules
    v[n,k] = sum_s c[n,s] u_hat[s,n,k]
    out = v / sqrt(sum_k v^2 + 1e-7)
    b[n,s] = sum_k out[n,k] u_hat[s,n,k]
Returns out [B, 5, 5].

Sharding: pure data-parallel over batch across 8 NeuronCores (16 batches
each); W replicated; no collectives.

Staging: the host feeds x pre-transposed per batch and cast to fp16 with
layout [b, p, db, s] (each SBUF partition line is one contiguous 6 KB
DMA chunk).  The device then needs NO PE transposes:

  per batch: DMA xT [128 d_lo, (db, s)] fp16; for (sh, db): matmul with
    the xT 128x128 chunk stationary and W[db] [128, 25] moving ->
    u_hat natural [128 s_lo, 25] accumulated over db in PSUM.
  per group of G batches: one PSUM->SBUF fp16 copy -> uh [128, G*4*25].

Routing (per group, natural layout): the s-partition sum AND the
broadcast of v to all partitions is ONE ones[128,128] matmul (pv128);
the squash norm runs replicated on all 128 partitions, with
rsqrt = Exp(-0.5*Ln(cs^2*s2 + eps) + ln(cs)) so the ACT engine only
ever uses the {exp, ln, square, copy} table set (zero table reloads).
Routing iterations of different groups are emitted wave- and
step-interleaved so their latency chains overlap on every engine.
"""

from contextlib import ExitStack
import math

import numpy as np

import concourse.bass as bass
import concourse.tile as tile
from concourse import mybir

F32 = mybir.dt.float32
F32R = mybir.dt.float32r
F16 = mybir.dt.float16
BF16 = mybir.dt.bfloat16
AX = mybir.AxisListType
OP = mybir.AluOpType
AF = mybir.ActivationFunctionType

N_CORES = 8
B_FULL, S, D = 128, 512, 768
NCAP, KDIM = 5, 5
NK = NCAP * KDIM  # 25
ROUTINGS = 4
T_EPS = 1e-7

ND = D // 128   # 6 d-blocks
NSB = S // 128  # 4 s-blocks (= s_hi)


class GroupState:
    def __init__(self, boff, G, pu):
        self.boff = boff
        self.G = G
        self.pu = pu
        self.uh16 = None  # fp16 SBUF copy of u_hat, made by iter0's s_t0
        self.blog = None  # logits tile, set by each iter for the next


def emit(ctx, tc, out, x, w, b_loc=16, group=4, warm=20):
    """Emit the per-core kernel IR.

    out: [1, b_loc*25] f32; x: [b_loc*128, 6*512] fp16 (pre-transposed,
    partition-major); w: [128, 6*25] fp16 (d_lo major, (db, nk) free).
    """
    nc = tc.nc
    groups = list(group) if isinstance(group, (list, tuple)) else \
        [group] * (b_loc // group)
    assert sum(groups) == b_loc
    NG = len(groups)

    const_pool = ctx.enter_context(tc.tile_pool(name="const", bufs=1))
    xt_pool = ctx.enter_context(tc.tile_pool(name="xt", bufs=16))
    pu_pool = ctx.enter_context(tc.tile_pool(name="pu", bufs=2, space="PSUM"))
    rt_pool = ctx.enter_context(tc.tile_pool(name="rt", bufs=3))
    pv_pool = ctx.enter_context(tc.tile_pool(name="pv", bufs=4, space="PSUM"))

    # --- first x DMAs before anything else on the sync ring ---
    xt_tiles = {}
    def emit_batch_dma(b):
        xt = xt_pool.tile([128, ND * S], F16, tag="xt", name=f"xt{b}")
        eng = nc.sync if (b % 2 == 0) else nc.scalar
        eng.dma_start(xt[:], x[b * 128:(b + 1) * 128, :])
        xt_tiles[b] = xt

    # --- constants (W first: every main matmul needs it) ---
    w_sb = const_pool.tile([128, ND * NK], F16)
    nc.sync.dma_start(w_sb[:], w)
    for _b in range(b_loc):
        emit_batch_dma(_b)
    ones_f = const_pool.tile([128, 128], F32)
    nc.gpsimd.memset(ones_f[:], 1.0)
    ones_sq = const_pool.tile([128, 128], F16)
    nc.vector.tensor_copy(ones_sq[:], ones_f[:])
    eps_b = const_pool.tile([128, 1], F32)
    nc.gpsimd.memset(eps_b[:], T_EPS)
    lncs_b = const_pool.tile([128, 1], F32)
    nc.gpsimd.memset(lncs_b[:], math.log(1.0 / NCAP))

    # Trigger the single ACT table load ({exp, ln, square, copy} set)
    # during the first DMA rather than on the routing critical path.
    dexp = const_pool.tile([1, 1], F32)
    nc.scalar.activation(dexp[:], ones_f[0:1, 0:1], AF.Exp)

    # HAM warm-up: regular fp16 matmuls during the first DMA so the PE
    # clock gate is at 8/8 when real work lands.
    wps = pv_pool.tile([1, 128], F32, tag="warm", bufs=1)
    for _ in range(warm):
        nc.tensor.matmul(wps[:], ones_sq[:, 0:1], ones_sq[:], start=True, stop=True)

    def emit_batch_mms(pu, bi, b):
        if b not in xt_tiles:
            emit_batch_dma(b)
        xt = xt_tiles.pop(b)
        for sh in range(NSB):
            col = (bi * NSB + sh) * NK
            for db in range(ND):
                nc.tensor.matmul(
                    pu[:, col:col + NK],
                    xt[:, db * S + sh * 128:db * S + sh * 128 + 128],
                    w_sb[:, db * NK:(db + 1) * NK],
                    start=(db == 0),
                    stop=(db == ND - 1),
                )

    def routing_iter_steps(st, it):
        """Step closures for one routing iteration of one group.

        Engine split per iter: ACT {exp, square, ln, exp(rsqrt)},
        DVE {den-reduce, recip, c, t, tmp, blog}, Pool {s2},
        PE {4 accumulating pv128 matmuls}.
        """
        G = st.G
        cs = 1.0 / NCAP if it == 0 else 1.0

        def uh_ap():
            return st.uh16[:].rearrange(
                "p (b sh n k) -> p b sh n k", b=G, sh=NSB, n=NCAP
            )
        steps = []
        box = {}

        if it > 0:
            def s_exp(blog=st.blog):
                expb = rt_pool.tile([128, G * NSB * NCAP], BF16, tag="expb")
                nc.scalar.activation(expb[:], blog[:], AF.Exp)
                box["expb"] = expb
            steps.append(s_exp)

            def s_den():
                den = rt_pool.tile([128, G * NSB], F32, tag="den")
                nc.vector.reduce_sum(
                    den[:],
                    box["expb"][:].rearrange("p (bs n) -> p bs n", n=NCAP),
                    axis=AX.X,
                )
                rden = rt_pool.tile([128, G * NSB], F32, tag="rden")
                nc.vector.reciprocal(rden[:], den[:])
                box["rden"] = rden
            steps.append(s_den)

            def s_c():
                c = rt_pool.tile([128, G * NSB * NCAP], F16, tag="c")
                nc.gpsimd.tensor_tensor(
                    c[:].rearrange("p (b sh n) -> p b sh n", b=G, sh=NSB),
                    box["expb"][:].rearrange("p (b sh n) -> p b sh n", b=G, sh=NSB),
                    box["rden"][:].rearrange("p (b sh) -> p b sh", b=G)
                    .unsqueeze(3)
                    .broadcast_to((128, G, NSB, NCAP)),
                    op=OP.mult,
                )
                box["c"] = c
            steps.append(s_c)

            def s_t():
                t = rt_pool.tile([128, G * NSB * NK], F16, tag="t")
                t_ap = t[:].rearrange(
                    "p (b sh n k) -> p b sh n k", b=G, sh=NSB, n=NCAP
                )
                nc.vector.tensor_tensor(
                    t_ap,
                    uh_ap(),
                    box["c"][:]
                    .rearrange("p (b sh n) -> p b sh n", b=G, sh=NSB)
                    .unsqueeze(4)
                    .broadcast_to((128, G, NSB, NCAP, KDIM)),
                    op=OP.mult,
                )
                box["t_ap"] = t_ap
            steps.append(s_t)
        else:
            def s_t0():
                uh16 = rt_pool.tile(
                    [128, G * NSB * NK], F16, tag="uh", bufs=5,
                    name=f"uh{st.boff}",
                )
                nc.vector.tensor_copy(uh16[:], st.pu[:])
                st.uh16 = uh16
                box["t_ap"] = uh16[:].rearrange(
                    "p (b sh n k) -> p b sh n k", b=G, sh=NSB, n=NCAP
                )
            steps.append(s_t0)

        def s_mm():
            # v replicated to all 128 partitions: ones[128,128] stationary,
            # accumulate the 4 s_hi blocks.
            pv = pv_pool.tile([128, G * NK], F32, tag="pv")
            for sh in range(NSB):
                nc.tensor.matmul(
                    pv[:],
                    ones_sq[:],
                    box["t_ap"][:, :, sh, :, :],
                    start=(sh == 0),
                    stop=(sh == NSB - 1),
                )
            box["pv"] = pv
        steps.append(s_mm)

        def s_sq():
            sq = rt_pool.tile([128, G * NK], F32, tag="sq")
            nc.scalar.activation(sq[:], box["pv"][:], AF.Square)
            box["sq"] = sq
        steps.append(s_sq)

        def s_s2():
            s2 = rt_pool.tile([128, G * NCAP], F32, tag="s2")
            nc.vector.reduce_sum(
                s2[:],
                box["sq"][:].rearrange("p (bn k) -> p bn k", k=KDIM),
                axis=AX.X,
            )
            box["s2"] = s2
        steps.append(s_s2)

        def s_rnrm():
            # rsqrt via the exp/ln table set: cs/sqrt(cs^2*s2 + eps)
            lns = rt_pool.tile([128, G * NCAP], F32, tag="lns")
            nc.scalar.activation(
                lns[:], box["s2"][:], AF.Ln, bias=eps_b[:], scale=cs * cs
            )
            rnrm = rt_pool.tile([128, G * NCAP], F32, tag="rnrm")
            if it == 0:
                nc.scalar.activation(
                    rnrm[:], lns[:], AF.Exp, bias=lncs_b[:], scale=-0.5
                )
            else:
                nc.scalar.activation(rnrm[:], lns[:], AF.Exp, scale=-0.5)
            box["rnrm"] = rnrm
        steps.append(s_rnrm)

        if it < ROUTINGS - 1:
            def s_tmp():
                tmp = rt_pool.tile([128, G * NSB * NK], F32, tag="tmp")
                nc.vector.tensor_tensor(
                    tmp[:].rearrange(
                        "p (b sh n k) -> p b sh n k", b=G, sh=NSB, n=NCAP
                    ),
                    uh_ap(),
                    box["pv"][:]
                    .rearrange("p (b n k) -> p b n k", b=G, n=NCAP)
                    .unsqueeze(2)
                    .broadcast_to((128, G, NSB, NCAP, KDIM)),
                    op=OP.mult,
                )
                box["tmp"] = tmp
            steps.append(s_tmp)

            def s_wt():
                wt = rt_pool.tile([128, G * NSB * NCAP], F32, tag="wt")
                nc.vector.reduce_sum(
                    wt[:],
                    box["tmp"][:].rearrange("p (bsn k) -> p bsn k", k=KDIM),
                    axis=AX.X,
                )
                box["wt"] = wt
            steps.append(s_wt)

            def s_blog():
                blog = rt_pool.tile([128, G * NSB * NCAP], F32, tag="blog")
                nc.gpsimd.tensor_tensor(
                    blog[:].rearrange("p (b sh n) -> p b sh n", b=G, sh=NSB),
                    box["wt"][:].rearrange("p (b sh n) -> p b sh n", b=G, sh=NSB),
                    box["rnrm"][:]
                    .rearrange("p (b n) -> p b n", b=G)
                    .unsqueeze(2)
                    .broadcast_to((128, G, NSB, NCAP)),
                    op=OP.mult,
                )
                st.blog = blog
            steps.append(s_blog)
        else:
            def s_out():
                outs = rt_pool.tile([1, G * NK], F32, tag="outs")
                nc.vector.tensor_tensor(
                    outs[:].rearrange("p (b n k) -> p b n k", b=G, n=NCAP),
                    box["pv"][0:1, :].rearrange("p (b n k) -> p b n k", b=G, n=NCAP),
                    box["rnrm"][0:1, :]
                    .rearrange("p (b n) -> p b n", b=G)
                    .unsqueeze(3)
                    .broadcast_to((1, G, NCAP, KDIM)),
                    op=OP.mult,
                )
                nc.sync.dma_start(
                    out[0:1, st.boff * NK:(st.boff + G) * NK], outs[0:1, :]
                )
            steps.append(s_out)
        return steps

    def emit_group_loads(boff, G):
        """Main matmuls for one group; u_hat accumulates in PSUM (pu).

        Each batch's matmuls are stamped with the batch's estimated DMA
        arrival time so the Tile scheduler (which does not model the
        DMA-paced reality) interleaves the per-engine streams correctly.
        """
        pu = pu_pool.tile([128, G * NSB * NK], F32, tag="pu", name=f"pu{boff}")
        for bi in range(G):
            b = boff + bi
            with tc.tile_wait_until(arrival(b)):
                emit_batch_mms(pu, bi, b)
        return GroupState(boff, G, pu)

    def arrival(b):
        # est. µs (as ms) when batch b's DMA completes: start ~9µs,
        # ~2µs of wire time per 786KB batch at ~400GB/s across both rings
        return (11.0 + 2.0 * b) / 1000.0

    # --- sequential emission; tile_wait_until stamps drive the global
    # software pipeline inside the Tile scheduler ---
    chains = []
    boff = 0
    for G in groups:
        chains.append(emit_group_loads(boff, G))
        boff += G
    for st in chains:
        for it in range(ROUTINGS):
            for fn in routing_iter_steps(st, it):
                fn()


def legalize_waits(nc):
    """This toolchain's walrus codegen accepts at most ONE sync wait per
    instruction ("Too many sync wait commands" otherwise) — and PE Matmult
    appears to take none safely. Hoist excess waits onto wait-only
    EventSemaphore instructions inserted just before, on the same engine
    (same pattern walrus already accepts for Tile's engine barriers)."""
    n = 0
    for fn in nc.m.functions:
        for blk in fn.blocks:
            new = []
            for inst in blk.instructions:
                si = inst.sync_info
                if si is not None and len(si.on_wait) > 0:
                    waits = list(si.on_wait)
                    keep = 0 if type(inst).__name__ == "InstMatmult" else 1
                    if len(waits) > keep:
                        for wt in waits[: len(waits) - keep]:
                            ev = mybir.InstEventSemaphore(
                                name=f"I-waitfix-{nc.next_id()}"
                            )
                            ev.engine = inst.engine
                            ev.sync_info = mybir.SyncInfo(on_wait=[wt], on_update=[])
                            new.append(ev)
                            n += 1
                        si.on_wait = waits[len(waits) - keep:]
                new.append(inst)
            blk.instructions = new
    return n


def build_caps_kernel(b_loc=16, group=4, warm=20):
    nc = bass.Bass(trn_type="TRN2", debug=False, target_bir_lowering=False)
    x = nc.dram_tensor("x", [b_loc * 128, ND * S], F16, kind="ExternalInput").ap()
    w = nc.dram_tensor("w", [128, ND * NK], F16, kind="ExternalInput").ap()
    out = nc.dram_tensor("out", [1, b_loc * NK], F32, kind="ExternalOutput").ap()
    with tile.TileContext(nc) as tc:
        with ExitStack() as ctx:
            emit(ctx, tc, out, x, w, b_loc=b_loc, group=group, warm=warm)
    legalize_waits(nc)
    return nc


_KERNEL_CFG = dict(group=(4, 4, 4, 2, 2), warm=20)


def prepare_in_maps(x: np.ndarray, W: np.ndarray, b_loc: int):
    """Shard + stage inputs: per core, x transposed to d-major per batch,
    laid out [b, p, db, s] (6 KB contiguous per partition) in fp16;
    W rearranged to [128, (db, nk)] fp16."""
    w16 = np.ascontiguousarray(
        W.reshape(ND, 128, NK).transpose(1, 0, 2).reshape(128, ND * NK)
    ).astype(np.float16)
    maps = []
    for i in range(N_CORES):
        xs = x[i * b_loc:(i + 1) * b_loc]  # [b_loc, S, D] f32
        # [b, S, D] -> [b, D, S] -> [b, db, p, s] -> [b, p, db, s]
        xt = (
            xs.transpose(0, 2, 1)
            .reshape(b_loc, ND, 128, S)
            .transpose(0, 2, 1, 3)
            .astype(np.float16)
        )
        maps.append({
            "x": np.ascontiguousarray(xt).reshape(b_loc * 128, ND * S),
            "w": w16,
        })
    return maps


def kernel(x: np.ndarray, W: np.ndarray) -> np.ndarray:
    from concourse.bass_utils import run_bass_kernel_spmd

    B, S_, D_ = x.shape
    assert (B, S_, D_) == (B_FULL, S, D)
    b_loc = B // N_CORES
    nc = build_caps_kernel(b_loc=b_loc, **_KERNEL_CFG)
    in_maps = prepare_in_maps(x, W, b_loc)
    res = run_bass_kernel_spmd(nc, in_maps, core_ids=list(range(N_CORES)))
    outs = [res.results[i]["out"].reshape(b_loc, NCAP, KDIM) for i in range(N_CORES)]
    return np.concatenate(outs, axis=0).astype(np.float32)


# revision 29
# speedup vs baseline: 1.5206x; 1.3168x over previous
"""Trainium2 Bass kernel for the capsule-routing layer (nn_Caps_Layer).

Computation (per batch b of x [B, S, D], W [D, 25]):
  u_hat = (x_b @ W).reshape(S, 5, 5)           # [S, n, k]
  b0 = 0;  for 4 routing iters:
    c = softmax_n(b)                            # over the 5 capsules
    v[n,k] = sum_s c[n,s] u_hat[s,n,k]
    out = v / sqrt(sum_k v^2 + 1e-7)
    b[n,s] = sum_k out[n,k] u_hat[s,n,k]
Returns out [B, 5, 5].

Sharding: pure data-parallel over batch across 8 NeuronCores (16 batches
each); W replicated; no collectives.

Staging: the host feeds x pre-transposed per batch and cast to fp16 with
layout [b, p, db, s] (each SBUF partition line is one contiguous 6 KB
DMA chunk).  The device then needs NO PE transposes:

  per batch: DMA xT [128 d_lo, (db, s)] fp16; for (sh, db): matmul with
    the xT 128x128 chunk stationary and W[db] [128, 25] moving ->
    u_hat natural [128 s_lo, 25] accumulated over db in PSUM.
  per group of G batches: one PSUM->SBUF fp16 copy -> uh [128, G*4*25].

Routing (per group, natural layout): the s-partition sum AND the
broadcast of v to all partitions is ONE ones[128,128] matmul (pv128);
the squash norm runs replicated on all 128 partitions, with
rsqrt = Exp(-0.5*Ln(cs^2*s2 + eps) + ln(cs)) so the ACT engine only
ever uses the {exp, ln, square, copy} table set (zero table reloads).
Routing iterations of different groups are emitted wave- and
step-interleaved so their latency chains overlap on every engine.
"""

from contextlib import ExitStack
import math

import numpy as np

import concourse.bass as bass
import concourse.tile as tile
from concourse import mybir

F32 = mybir.dt.float32
F32R = mybir.dt.float32r
F16 = mybir.dt.float16
BF16 = mybir.dt.bfloat16
AX = mybir.AxisListType
OP = mybir.AluOpType
AF = mybir.ActivationFunctionType

N_CORES = 8
B_FULL, S, D = 128, 512, 768
NCAP, KDIM = 5, 5
NK = NCAP * KDIM  # 25
ROUTINGS = 4
T_EPS = 1e-7

ND = D // 128   # 6 d-blocks
NSB = S // 128  # 4 s-blocks (= s_hi)


class GroupState:
    def __init__(self, boff, G, pu):
        self.boff = boff
        self.G = G
        self.pu = pu
        self.uh16 = None  # fp16 SBUF copy of u_hat, made by iter0's s_t0
        self.blog = None  # logits tile, set by each iter for the next


def emit(ctx, tc, out, x, w, b_loc=16, group=4, warm=20):
    """Emit the per-core kernel IR.

    out: [1, b_loc*25] f32; x: [b_loc*128, 6*512] fp16 (pre-transposed,
    partition-major); w: [128, 6*25] fp16 (d_lo major, (db, nk) free).
    """
    nc = tc.nc
    groups = list(group) if isinstance(group, (list, tuple)) else \
        [group] * (b_loc // group)
    assert sum(groups) == b_loc
    NG = len(groups)

    const_pool = ctx.enter_context(tc.tile_pool(name="const", bufs=1))
    xt_pool = ctx.enter_context(tc.tile_pool(name="xt", bufs=16))
    pu_pool = ctx.enter_context(tc.tile_pool(name="pu", bufs=2, space="PSUM"))
    rt_pool = ctx.enter_context(tc.tile_pool(name="rt", bufs=3))
    pv_pool = ctx.enter_context(tc.tile_pool(name="pv", bufs=4, space="PSUM"))

    # --- first x DMAs before anything else on the sync ring ---
    xt_tiles = {}
    def emit_batch_dma(b):
        xt = xt_pool.tile([128, ND * S], F16, tag="xt", name=f"xt{b}")
        eng = nc.sync if (b % 2 == 0) else nc.scalar
        eng.dma_start(xt[:], x[b * 128:(b + 1) * 128, :])
        xt_tiles[b] = xt

    # --- constants (W first: every main matmul needs it) ---
    w_sb = const_pool.tile([128, ND * NK], F16)
    nc.sync.dma_start(w_sb[:], w)
    for _b in range(b_loc):
        emit_batch_dma(_b)
    ones_f = const_pool.tile([128, 128], F32)
    nc.gpsimd.memset(ones_f[:], 1.0)
    ones_sq = const_pool.tile([128, 128], F16)
    nc.vector.tensor_copy(ones_sq[:], ones_f[:])
    eps_b = const_pool.tile([128, 1], F32)
    nc.gpsimd.memset(eps_b[:], T_EPS)
    lncs_b = const_pool.tile([128, 1], F32)
    nc.gpsimd.memset(lncs_b[:], math.log(1.0 / NCAP))

    # Trigger the single ACT table load ({exp, ln, square, copy} set)
    # during the first DMA rather than on the routing critical path.
    dexp = const_pool.tile([1, 1], F32)
    nc.scalar.activation(dexp[:], ones_f[0:1, 0:1], AF.Exp)

    # HAM warm-up: regular fp16 matmuls during the first DMA so the PE
    # clock gate is at 8/8 when real work lands.
    wps = pv_pool.tile([1, 128], F32, tag="warm", bufs=1)
    for _ in range(warm):
        nc.tensor.matmul(wps[:], ones_sq[:, 0:1], ones_sq[:], start=True, stop=True)

    def emit_batch_mms(pu, bi, b):
        if b not in xt_tiles:
            emit_batch_dma(b)
        xt = xt_tiles.pop(b)
        for sh in range(NSB):
            col = (bi * NSB + sh) * NK
            for db in range(ND):
                nc.tensor.matmul(
                    pu[:, col:col + NK],
                    xt[:, db * S + sh * 128:db * S + sh * 128 + 128],
                    w_sb[:, db * NK:(db + 1) * NK],
                    start=(db == 0),
                    stop=(db == ND - 1),
                )

    def routing_iter_steps(st, it):
        """Step closures for one routing iteration of one group.

        Engine split per iter: ACT {exp, square, ln, exp(rsqrt)},
        DVE {den-reduce, recip, c, t, tmp, blog}, Pool {s2},
        PE {4 accumulating pv128 matmuls}.
        """
        G = st.G
        cs = 1.0 / NCAP if it == 0 else 1.0

        def uh_ap():
            return st.uh16[:].rearrange(
                "p (b sh n k) -> p b sh n k", b=G, sh=NSB, n=NCAP
            )
        steps = []
        box = {}

        if it > 0:
            def s_exp(blog=st.blog):
                expb = rt_pool.tile([128, G * NSB * NCAP], BF16, tag="expb")
                nc.scalar.activation(expb[:], blog[:], AF.Exp)
                box["expb"] = expb
            steps.append(s_exp)

            def s_den():
                den = rt_pool.tile([128, G * NSB], F32, tag="den")
                nc.vector.reduce_sum(
                    den[:],
                    box["expb"][:].rearrange("p (bs n) -> p bs n", n=NCAP),
                    axis=AX.X,
                )
                rden = rt_pool.tile([128, G * NSB], F32, tag="rden")
                nc.vector.reciprocal(rden[:], den[:])
                box["rden"] = rden
            steps.append(s_den)

            def s_c():
                c = rt_pool.tile([128, G * NSB * NCAP], F16, tag="c")
                nc.gpsimd.tensor_tensor(
                    c[:].rearrange("p (b sh n) -> p b sh n", b=G, sh=NSB),
                    box["expb"][:].rearrange("p (b sh n) -> p b sh n", b=G, sh=NSB),
                    box["rden"][:].rearrange("p (b sh) -> p b sh", b=G)
                    .unsqueeze(3)
                    .broadcast_to((128, G, NSB, NCAP)),
                    op=OP.mult,
                )
                box["c"] = c
            steps.append(s_c)

            def s_t():
                t = rt_pool.tile([128, G * NSB * NK], F16, tag="t")
                t_ap = t[:].rearrange(
                    "p (b sh n k) -> p b sh n k", b=G, sh=NSB, n=NCAP
                )
                nc.vector.tensor_tensor(
                    t_ap,
                    uh_ap(),
                    box["c"][:]
                    .rearrange("p (b sh n) -> p b sh n", b=G, sh=NSB)
                    .unsqueeze(4)
                    .broadcast_to((128, G, NSB, NCAP, KDIM)),
                    op=OP.mult,
                )
                box["t_ap"] = t_ap
            steps.append(s_t)
        else:
            def s_t0():
                uh16 = rt_pool.tile(
                    [128, G * NSB * NK], F16, tag="uh", bufs=5,
                    name=f"uh{st.boff}",
                )
                nc.vector.tensor_copy(uh16[:], st.pu[:])
                st.uh16 = uh16
                box["t_ap"] = uh16[:].rearrange(
                    "p (b sh n k) -> p b sh n k", b=G, sh=NSB, n=NCAP
                )
            steps.append(s_t0)

        def s_mm():
            # v replicated to all 128 partitions: ones[128,128] stationary,
            # accumulate the 4 s_hi blocks.
            pv = pv_pool.tile([128, G * NK], F32, tag="pv")
            for sh in range(NSB):
                nc.tensor.matmul(
                    pv[:],
                    ones_sq[:],
                    box["t_ap"][:, :, sh, :, :],
                    start=(sh == 0),
                    stop=(sh == NSB - 1),
                )
            box["pv"] = pv
        steps.append(s_mm)

        def s_sq():
            sq = rt_pool.tile([128, G * NK], F32, tag="sq")
            nc.scalar.activation(sq[:], box["pv"][:], AF.Square)
            box["sq"] = sq
        steps.append(s_sq)

        def s_s2():
            s2 = rt_pool.tile([128, G * NCAP], F32, tag="s2")
            nc.vector.reduce_sum(
                s2[:],
                box["sq"][:].rearrange("p (bn k) -> p bn k", k=KDIM),
                axis=AX.X,
            )
            box["s2"] = s2
        steps.append(s_s2)

        def s_rnrm():
            # rsqrt via the exp/ln table set: cs/sqrt(cs^2*s2 + eps)
            lns = rt_pool.tile([128, G * NCAP], F32, tag="lns")
            nc.scalar.activation(
                lns[:], box["s2"][:], AF.Ln, bias=eps_b[:], scale=cs * cs
            )
            rnrm = rt_pool.tile([128, G * NCAP], F32, tag="rnrm")
            if it == 0:
                nc.scalar.activation(
                    rnrm[:], lns[:], AF.Exp, bias=lncs_b[:], scale=-0.5
                )
            else:
                nc.scalar.activation(rnrm[:], lns[:], AF.Exp, scale=-0.5)
            box["rnrm"] = rnrm
        steps.append(s_rnrm)

        if it < ROUTINGS - 1:
            def s_tmp():
                tmp = rt_pool.tile([128, G * NSB * NK], F32, tag="tmp")
                nc.vector.tensor_tensor(
                    tmp[:].rearrange(
                        "p (b sh n k) -> p b sh n k", b=G, sh=NSB, n=NCAP
                    ),
                    uh_ap(),
                    box["pv"][:]
                    .rearrange("p (b n k) -> p b n k", b=G, n=NCAP)
                    .unsqueeze(2)
                    .broadcast_to((128, G, NSB, NCAP, KDIM)),
                    op=OP.mult,
                )
                box["tmp"] = tmp
            steps.append(s_tmp)

            def s_wt():
                wt = rt_pool.tile([128, G * NSB * NCAP], F32, tag="wt")
                nc.vector.reduce_sum(
                    wt[:],
                    box["tmp"][:].rearrange("p (bsn k) -> p bsn k", k=KDIM),
                    axis=AX.X,
                )
                box["wt"] = wt
            steps.append(s_wt)

            def s_blog():
                blog = rt_pool.tile([128, G * NSB * NCAP], F32, tag="blog")
                nc.gpsimd.tensor_tensor(
                    blog[:].rearrange("p (b sh n) -> p b sh n", b=G, sh=NSB),
                    box["wt"][:].rearrange("p (b sh n) -> p b sh n", b=G, sh=NSB),
                    box["rnrm"][:]
                    .rearrange("p (b n) -> p b n", b=G)
                    .unsqueeze(2)
                    .broadcast_to((128, G, NSB, NCAP)),
                    op=OP.mult,
                )
                st.blog = blog
            steps.append(s_blog)
        else:
            def s_out():
                outs = rt_pool.tile([1, G * NK], F32, tag="outs")
                nc.vector.tensor_tensor(
                    outs[:].rearrange("p (b n k) -> p b n k", b=G, n=NCAP),
                    box["pv"][0:1, :].rearrange("p (b n k) -> p b n k", b=G, n=NCAP),
                    box["rnrm"][0:1, :]
                    .rearrange("p (b n) -> p b n", b=G)
                    .unsqueeze(3)
                    .broadcast_to((1, G, NCAP, KDIM)),
                    op=OP.mult,
                )
                nc.sync.dma_start(
                    out[0:1, st.boff * NK:(st.boff + G) * NK], outs[0:1, :]
                )
            steps.append(s_out)
        return steps

    def emit_group_loads(boff, G):
        """Main matmuls for one group; u_hat accumulates in PSUM (pu).

        Each batch's matmuls are stamped with the batch's estimated DMA
        arrival time so the Tile scheduler (which does not model the
        DMA-paced reality) interleaves the per-engine streams correctly.
        """
        pu = pu_pool.tile([128, G * NSB * NK], F32, tag="pu", name=f"pu{boff}")
        for bi in range(G):
            b = boff + bi
            with tc.tile_wait_until(arrival(b)):
                emit_batch_mms(pu, bi, b)
        return GroupState(boff, G, pu)

    def arrival(b):
        # est. µs (as ms) when batch b's DMA completes: start ~9µs,
        # ~2µs of wire time per 786KB batch at ~400GB/s across both rings
        return (11.0 + 2.0 * b) / 1000.0

    # --- emission: loads stamped with DMA-arrival estimates; routing
    # waves step-interleaved so tail chains overlap (emission priority
    # tie-breaks the scheduler once sim-times are equal) ---
    b_offs = []
    o = 0
    for G in groups:
        b_offs.append(o)
        o += G
    states = [emit_group_loads(b_offs[g], groups[g]) for g in range(NG)]
    for wv in range(NG + ROUTINGS - 1):
        wave = []
        for g in range(NG):
            it = wv - g
            if 0 <= it < ROUTINGS:
                wave.append(routing_iter_steps(states[g], it))
        for j in range(max(len(ch) for ch in wave)):
            for ch in wave:
                if j < len(ch):
                    ch[j]()


def legalize_waits(nc):
    """This toolchain's walrus codegen accepts at most ONE sync wait per
    instruction ("Too many sync wait commands" otherwise) — and PE Matmult
    appears to take none safely. Hoist excess waits onto wait-only
    EventSemaphore instructions inserted just before, on the same engine
    (same pattern walrus already accepts for Tile's engine barriers)."""
    n = 0
    for fn in nc.m.functions:
        for blk in fn.blocks:
            new = []
            for inst in blk.instructions:
                si = inst.sync_info
                if si is not None and len(si.on_wait) > 0:
                    waits = list(si.on_wait)
                    keep = 0 if type(inst).__name__ == "InstMatmult" else 1
                    if len(waits) > keep:
                        for wt in waits[: len(waits) - keep]:
                            ev = mybir.InstEventSemaphore(
                                name=f"I-waitfix-{nc.next_id()}"
                            )
                            ev.engine = inst.engine
                            ev.sync_info = mybir.SyncInfo(on_wait=[wt], on_update=[])
                            new.append(ev)
                            n += 1
                        si.on_wait = waits[len(waits) - keep:]
                new.append(inst)
            blk.instructions = new
    return n


def build_caps_kernel(b_loc=16, group=4, warm=20):
    nc = bass.Bass(trn_type="TRN2", debug=False, target_bir_lowering=False)
    x = nc.dram_tensor("x", [b_loc * 128, ND * S], F16, kind="ExternalInput").ap()
    w = nc.dram_tensor("w", [128, ND * NK], F16, kind="ExternalInput").ap()
    out = nc.dram_tensor("out", [1, b_loc * NK], F32, kind="ExternalOutput").ap()
    with tile.TileContext(nc) as tc:
        with ExitStack() as ctx:
            emit(ctx, tc, out, x, w, b_loc=b_loc, group=group, warm=warm)
    legalize_waits(nc)
    return nc


_KERNEL_CFG = dict(group=(4, 4, 4, 2, 2), warm=20)


def prepare_in_maps(x: np.ndarray, W: np.ndarray, b_loc: int):
    """Shard + stage inputs: per core, x transposed to d-major per batch,
    laid out [b, p, db, s] (6 KB contiguous per partition) in fp16;
    W rearranged to [128, (db, nk)] fp16."""
    w16 = np.ascontiguousarray(
        W.reshape(ND, 128, NK).transpose(1, 0, 2).reshape(128, ND * NK)
    ).astype(np.float16)
    maps = []
    for i in range(N_CORES):
        xs = x[i * b_loc:(i + 1) * b_loc]  # [b_loc, S, D] f32
        # [b, S, D] -> [b, D, S] -> [b, db, p, s] -> [b, p, db, s]
        xt = (
            xs.transpose(0, 2, 1)
            .reshape(b_loc, ND, 128, S)
            .transpose(0, 2, 1, 3)
            .astype(np.float16)
        )
        maps.append({
            "x": np.ascontiguousarray(xt).reshape(b_loc * 128, ND * S),
            "w": w16,
        })
    return maps


def kernel(x: np.ndarray, W: np.ndarray) -> np.ndarray:
    from concourse.bass_utils import run_bass_kernel_spmd

    B, S_, D_ = x.shape
    assert (B, S_, D_) == (B_FULL, S, D)
    b_loc = B // N_CORES
    nc = build_caps_kernel(b_loc=b_loc, **_KERNEL_CFG)
    in_maps = prepare_in_maps(x, W, b_loc)
    res = run_bass_kernel_spmd(nc, in_maps, core_ids=list(range(N_CORES)))
    outs = [res.results[i]["out"].reshape(b_loc, NCAP, KDIM) for i in range(N_CORES)]
    return np.concatenate(outs, axis=0).astype(np.float32)
